# revision 26
# baseline (speedup 1.0000x reference)
"""Trainium2 kernel for nn_Net_57277683859526 (batched tiny-MLP ensemble).

E=256 independent MLPs (15 -> 128 -> 128 -> 1, sigmoid) over shared x[8192, 15].
Expert-parallel on 8 NeuronCores: 32 experts/core, full batch per core.

All activations are computed in tanh half-angle form, t = tanh((z+b)/2), with
sigma(z) = (1+t)/2 folded into the next layer's weights on the host:
  W2 -> 0.5*W2,  b2 -> b2 + 0.5*colsum(W2);  W3 -> 0.5*W3, b3 += 0.5*sum(W3).
This makes both activation engines usable:
  - ScalarE (ACT): exact Tanh, with the free per-instruction scale/bias.
  - VectorE (DVE): a runtime-registered custom 8-stage op TANH5C_ANT computing
    p(clamp(u)) = u*(c0 + u^2*(c1 + u^2*c2)) ~ tanh(Z*u/2) at 1 elem/cyc/lane.
L1 preacts are pre-scaled by 1/Z (Z=4.5) with the bias as a 16th contraction
row, so the DVE op needs no per-expert constants and handles ~13/14 of L1;
ACT handles the rest of L1 plus all of L2 (per-expert bias rides the ACT
instruction).

Pipeline (per core): 256 (chunk, expert) iterations at CH=1024, skewed so
PE / DVE / ACT stream concurrently. PSUM (8 banks): ps1/ps2 rotate through one
3-buffer [128,1024] fp32 pool (6 banks); L3 accumulates all 32 experts into one
[128,1024] tile per chunk (2 banks) via zero-padded per-expert W3 column tiles
at partition 32*(e%4) + e//4 (zero-init matmul sets has_written so every expert
MM accumulates), copied out once per chunk. L1 uses full 128-row contraction
(zero-padded weights; same cost since PE time = moving cols) because partial
row-group matmuls do not register as PE activity for the HAM clock-gate and
left the PE throttled at 1.2 GHz; L2 uses 4x col-tiled strips that run
concurrently on the PE array.
"""

import numpy as np
import ml_dtypes

DIM = 16
E = DIM * DIM          # 256 experts
D_IN = DIM - 1         # 15
H = 128
B = 8192
N_CORES = 8
E_CORE = E // N_CORES  # 32
CH = 1024              # batch chunk (tile free dim)
NCH = B // CH          # 8
SUBW = 512             # matmul moving width (1 fp32 PSUM bank)
Z = 4.5                # L1 preact pre-scale (clamp point of the DVE poly)
PC0, PC1, PC2 = 2.08635576, -2.07713538, 0.98841492  # tanh(Z*u/2) poly coeffs

_prog_cache = {}
L3_MODE = "accum"  # "accum" (packed 32-expert psum accumulate) | "psc" (debug: per-quad single-row)


def _register_tanh5c():
    """Register the custom DVE op (idempotent)."""
    from concourse import dve_ops
    from concourse.dve_spec import Spec, Src0, C0, C1, C2, Zero, One, maxx, minn, lower
    from concourse.dve_uop import DveOpSpec

    name = "TANH5C_ANT"
    if name in dve_ops._SUB_OPCODE_FOR_NAME:
        return next(op for op in dve_ops.OPS if op.name == name)

    xc = maxx(minn(Src0, One), Zero - One)
    s = xc * xc
    body = ((s * C2 + C1) * s + C0) * xc

    def _ref(in0, in1, s0, s1, imm2):
        u = np.clip(in0.astype(np.float32), -1.0, 1.0)
        sq = u * u
        return ((sq * imm2 + s1) * sq + s0) * u

    spec = Spec(body=body, reference=_ref)
    row = max(dve_ops._SUB_OPCODE_FOR_NAME.values()) + 1
    dve_ops._SUB_OPCODE_FOR_NAME[name] = row
    shas = {}
    for ver in ("v3", "v4"):
        uops = lower(spec, ver=ver)
        shas[ver] = DveOpSpec(name=name, opcode=row, uops=uops, rd1_en=False).sha(ver)
    op = dve_ops.DveOp(name, spec, subdim=False, uops_sha=shas)
    dve_ops.OPS.append(op)
    dve_ops.CUSTOM_DVE_SPECS[name] = spec
    return op


def _act1_on_act(e, c):
    """Which L1 tiles go to ScalarE (exact tanh) instead of the DVE poly."""
    return ((e + 3 * c) % 16) == 0


def _build_program():
    if "nc" in _prog_cache:
        return _prog_cache["nc"]

    import concourse.mybir as mybir
    import concourse.tile as tile
    from concourse import bacc

    F32 = mybir.dt.float32
    BF16 = mybir.dt.bfloat16
    TANH = mybir.ActivationFunctionType.Tanh
    dveop = _register_tanh5c()

    nc = bacc.Bacc()

    xT = nc.declare_dram_parameter("xT", [128, B], BF16, isOutput=False)
    w1p = nc.declare_dram_parameter("w1p", [128, E_CORE * H], BF16, isOutput=False)
    w2p = nc.declare_dram_parameter("w2p", [H, E_CORE * H], BF16, isOutput=False)
    w3p = nc.declare_dram_parameter("w3p", [H, E_CORE * 32], BF16, isOutput=False)
    c2p = nc.declare_dram_parameter("c2p", [H, E_CORE], F32, isOutput=False)
    out = nc.declare_dram_parameter("out", [E_CORE, B], BF16, isOutput=True)

    with tile.TileContext(nc) as tc:
        with (
            tc.tile_pool(name="const", bufs=1) as const,
            tc.tile_pool(name="t1p", bufs=5) as t1p,
            tc.tile_pool(name="t2p", bufs=8) as t2p,
            tc.tile_pool(name="ocp", bufs=3) as ocp,
            tc.tile_pool(name="pp", bufs=3, space="PSUM") as pp,
            tc.tile_pool(name="p3", bufs=1, space="PSUM") as p3,
        ):
            xT_s = const.tile([128, B], BF16, tag="xT")
            w1s = const.tile([128, E_CORE * H], BF16, tag="w1")
            w2s = const.tile([H, E_CORE * H], BF16, tag="w2")
            w3s = const.tile([H, E_CORE * 32], BF16, tag="w3")
            c2s = const.tile([H, E_CORE], F32, tag="c2")
            zrow = const.tile([1, H], BF16, tag="zrow")
            nc.vector.memset(zrow[:], 0.0)
            # priority-ordered input loads: what iteration 0 needs goes first
            nc.sync.dma_start(out=w1s[:, 0:4 * H], in_=w1p[:, 0:4 * H])
            nc.sync.dma_start(out=xT_s[:, 0:CH], in_=xT[:, 0:CH])
            nc.sync.dma_start(out=w2s[:, 0:2 * H], in_=w2p[:, 0:2 * H])
            nc.sync.dma_start(out=c2s[:], in_=c2p[:])
            nc.sync.dma_start(out=w3s[:], in_=w3p[:])
            for q in range(NCH - 1):
                sl = slice(CH + q * CH, CH + (q + 1) * CH)
                nc.sync.dma_start(out=xT_s[:, sl], in_=xT[:, sl])
            nc.sync.dma_start(out=w1s[:, 4 * H:], in_=w1p[:, 4 * H:])
            for q in range(5):
                lo = 2 * H + q * 6 * H
                hi = min(lo + 6 * H, E_CORE * H)
                nc.sync.dma_start(out=w2s[:, lo:hi], in_=w2p[:, lo:hi])

            NK = NCH * E_CORE  # 128 linear (chunk, expert) iterations
            ps1 = {}           # k -> psum tile with L1 preacts (z1/Z, bf16)
            ps2 = {}           # k -> psum tile with L2 preacts (z2', bf16)
            t1 = {}
            t2 = {}
            ps3 = None

            for k in range(NK + 2):
                c, e = k // E_CORE, k % E_CORE
                # --- L1 matmuls for iter k, first on the PE queue ---
                if k < NK:
                    p = pp.tile([128, CH], F32, tag="ps")
                    # full 128-row contraction (rows 16..127 are zero): same
                    # cost (PE time = moving cols) but counts as full PE
                    # activity for the HAM clock-gate, keeping the PE at 2.4GHz
                    for s2_ in range(2):
                        nc.tensor.matmul(
                            p[:, SUBW * s2_:SUBW * (s2_ + 1)],
                            w1s[:, H * e:H * (e + 1)],
                            xT_s[:, CH * c + SUBW * s2_:CH * c + SUBW * (s2_ + 1)],
                            start=True,
                            stop=True,
                        )
                    ps1[k] = p
                # --- act2 for iter k-2 (leads the ACT queue: it frees the
                # pool slot that iter k's L1 reuses) ---
                k2 = k - 2
                quad_ready = False
                if 0 <= k2 < NK:
                    c2, e2 = k2 // E_CORE, k2 % E_CORE
                    t = t2p.tile([128, CH], BF16, tag="t2")
                    nc.scalar.activation(
                        t[:], ps2[k2][:], TANH,
                        bias=c2s[:, e2:e2 + 1], scale=0.5,
                    )
                    t2[k2] = t
                    del ps2[k2]
                    quad_ready = (e2 % 4 == 3)
                # --- act1 for iter k-1 ---
                k1 = k - 1
                if 0 <= k1 < NK:
                    c1, e1 = k1 // E_CORE, k1 % E_CORE
                    t = t1p.tile([128, CH], BF16, tag="t1")
                    if _act1_on_act(e1, c1):
                        nc.scalar.activation(t[:], ps1[k1][:], TANH, scale=Z / 2)
                    else:
                        nc.vector._custom_dve(
                            dveop, out=t[:], in0=ps1[k1][:],
                            s0=PC0, s1=PC1, imm2=PC2,
                        )
                    t1[k1] = t
                    del ps1[k1]
                    # L2 matmuls (col-tiled 4x: strips run concurrently)
                    p = pp.tile([128, CH], F32, tag="ps")
                    for s2_ in range(2):
                        for j in range(4):
                            nc.tensor.matmul(
                                p[32 * j:32 * (j + 1),
                                  SUBW * s2_:SUBW * (s2_ + 1)],
                                w2s[:, H * e1 + 32 * j:H * e1 + 32 * (j + 1)],
                                t[:, SUBW * s2_:SUBW * (s2_ + 1)],
                                start=True,
                                stop=True,
                                tile_position=(0, 32 * j),
                            )
                    ps2[k1] = p
                # --- L3 quad (after L2 so PE stays ordered) ---
                if quad_ready:
                    if ps3 is None:
                        ps3 = p3.tile([128, CH], F32, tag="ps3")
                        for s2_ in range(2):
                            # zero-init: sets has_written for the whole
                            # region so all expert MMs accumulate
                            nc.tensor.matmul(
                                ps3[:, SUBW * s2_:SUBW * (s2_ + 1)],
                                zrow[:],
                                xT_s[0:1, SUBW * s2_:SUBW * (s2_ + 1)],
                                start=True,
                                stop=False,
                            )
                    for s2_ in range(2):
                        for eq in range(e2 - 3, e2 + 1):
                            j = eq % 4
                            nc.tensor.matmul(
                                ps3[32 * j:32 * j + 32,
                                    SUBW * s2_:SUBW * (s2_ + 1)],
                                w3s[:, 32 * eq:32 * (eq + 1)],
                                t2[k2 - 3 + (eq - (e2 - 3))][
                                    :, SUBW * s2_:SUBW * (s2_ + 1)],
                                start=False,
                                stop=(eq == E_CORE - 1),
                                tile_position=(0, 32 * j),
                            )
                    for eq in range(e2 - 3, e2 + 1):
                        del t2[k2 - 3 + (eq - (e2 - 3))]
                    if e2 == E_CORE - 1:
                        o = ocp.tile([128, CH], BF16, tag="oc")
                        nc.vector.tensor_copy(o[:], ps3[:])
                        ps3 = None
                        for j in range(4):
                            nc.sync.dma_start(
                                out=out[8 * j:8 * j + 8,
                                        CH * c2:CH * (c2 + 1)],
                                in_=o[32 * j:32 * j + 8, :],
                            )
    nc.finalize()
    _prog_cache["nc"] = nc
    return nc


def _prep_inputs(x_batch, W1, b1, W2, b2, W3):
    """Host-side shard + layout prep. Returns per-core input maps."""
    bf = ml_dtypes.bfloat16
    # x~ = [x; 1] transposed, replicated in 4 row-bands of 32
    xT1 = np.concatenate(
        [x_batch.T, np.ones((1, B), np.float32)], axis=0
    )  # [16, B]
    xT = np.zeros((128, B), dtype=bf)
    for kb in range(4):
        xT[32 * kb:32 * kb + D_IN + 1] = xT1.astype(bf)
    in_maps = []
    for cid in range(N_CORES):
        sl = slice(cid * E_CORE, (cid + 1) * E_CORE)
        W1c, b1c = W1[sl], b1[sl]          # [32,15,128], [32,128]
        W2c, b2c = W2[sl], b2[sl]          # [32,128,128], [32,128]
        W3c = W3[sl, :, 0]                 # [32,128]
        w1pk = np.zeros((128, E_CORE * H), dtype=bf)
        for e in range(E_CORE):
            blk = (np.concatenate(
                [W1c[e], b1c[e][None, :]], axis=0
            ) / Z).astype(bf)  # [16,128]
            w1pk[0:16, H * e:H * (e + 1)] = blk
        w2pk = np.ascontiguousarray(
            (0.5 * W2c).transpose(1, 0, 2).reshape(H, E_CORE * H)
        ).astype(bf)
        c2pk = np.ascontiguousarray(
            ((b2c + 0.5 * W2c.sum(axis=1)) / 2.0).T
        ).astype(np.float32)               # [128, 32]
        w3pk = np.zeros((H, E_CORE * 32), dtype=bf)
        for e in range(E_CORE):
            w3pk[:, 32 * e + e // 4] = (0.5 * W3c[e]).astype(bf)
        in_maps.append(
            {"xT": xT, "w1p": w1pk, "w2p": w2pk, "w3p": w3pk, "c2p": c2pk}
        )
    return in_maps


def run(x_batch, W1, b1, W2, b2, W3, b3, trace=False):
    """Run on 8 NeuronCores; returns (output [B,16,16] f32, BassKernelResults)."""
    from concourse.bass_utils import run_bass_kernel_spmd

    nc = _build_program()
    x_batch = np.asarray(x_batch, dtype=np.float32)
    W1 = np.asarray(W1, dtype=np.float32)
    b1 = np.asarray(b1, dtype=np.float32)
    W2 = np.asarray(W2, dtype=np.float32)
    b2 = np.asarray(b2, dtype=np.float32)
    W3 = np.asarray(W3, dtype=np.float32)
    b3 = np.asarray(b3, dtype=np.float32)
    in_maps = _prep_inputs(x_batch, W1, b1, W2, b2, W3)
    res = run_bass_kernel_spmd(
        nc, in_maps, core_ids=list(range(N_CORES)), trace=trace
    )
    # device row 8*(e%4) + e//4  ->  expert e
    if L3_MODE == "psc":
        perm = np.arange(E_CORE)
    else:
        perm = np.array([8 * (e % 4) + e // 4 for e in range(E_CORE)])
    outs = []
    for cid in range(N_CORES):
        o = np.asarray(res.results[cid]["out"]).astype(np.float32)[perm]
        outs.append(o)
    out_full = np.concatenate(outs, axis=0)  # [E, B]
    c3 = b3[:, 0] + 0.5 * W3[:, :, 0].sum(axis=1)
    out_full = out_full + c3[:, None]
    return out_full.T.reshape(B, DIM, DIM).astype(np.float32), res


def kernel(x_batch, W1, b1, W2, b2, W3, b3):
    out, _ = run(x_batch, W1, b1, W2, b2, W3, b3, trace=False)
    return out


if __name__ == "__main__":
    rng = np.random.default_rng(0)
    ins = {
        "x_batch": rng.standard_normal((B, D_IN)).astype(np.float32),
        "W1": (rng.standard_normal((E, D_IN, H)) / np.sqrt(D_IN)).astype(np.float32),
        "b1": (rng.standard_normal((E, H)) / np.sqrt(D_IN)).astype(np.float32),
        "W2": (rng.standard_normal((E, H, H)) / np.sqrt(H)).astype(np.float32),
        "b2": (rng.standard_normal((E, H)) / np.sqrt(H)).astype(np.float32),
        "W3": (rng.standard_normal((E, H, 1)) / np.sqrt(H)).astype(np.float32),
        "b3": (rng.standard_normal((E, 1)) / np.sqrt(H)).astype(np.float32),
    }
    out = kernel(**ins)
    print("kernel ran, out shape:", out.shape, out.dtype)


# revision 27
# speedup vs baseline: 1.0200x; 1.0200x over previous
"""Trainium2 kernel for nn_Net_57277683859526 (batched tiny-MLP ensemble).

E=256 independent MLPs (15 -> 128 -> 128 -> 1, sigmoid) over shared x[8192, 15].
Expert-parallel on 8 NeuronCores: 32 experts/core, full batch per core.

All activations are computed in tanh half-angle form, t = tanh((z+b)/2), with
sigma(z) = (1+t)/2 folded into the next layer's weights on the host:
  W2 -> 0.5*W2,  b2 -> b2 + 0.5*colsum(W2);  W3 -> 0.5*W3, b3 += 0.5*sum(W3).
This makes both activation engines usable:
  - ScalarE (ACT): exact Tanh, with the free per-instruction scale/bias.
  - VectorE (DVE): a runtime-registered custom 8-stage op TANH5C_ANT computing
    p(clamp(u)) = u*(c0 + u^2*(c1 + u^2*c2)) ~ tanh(Z*u/2) at 1 elem/cyc/lane.
L1 preacts are pre-scaled by 1/Z (Z=4.5) with the bias as a 16th contraction
row, so the DVE op needs no per-expert constants and handles ~13/14 of L1;
ACT handles the rest of L1 plus all of L2 (per-expert bias rides the ACT
instruction).

Pipeline (per core): 256 (chunk, expert) iterations at CH=1024, skewed so
PE / DVE / ACT stream concurrently. PSUM (8 banks): ps1/ps2 rotate through one
3-buffer [128,1024] fp32 pool (6 banks); L3 accumulates all 32 experts into one
[128,1024] tile per chunk (2 banks) via zero-padded per-expert W3 column tiles
at partition 32*(e%4) + e//4 (zero-init matmul sets has_written so every expert
MM accumulates), copied out once per chunk. L1 uses full 128-row contraction
(zero-padded weights; same cost since PE time = moving cols) because partial
row-group matmuls do not register as PE activity for the HAM clock-gate and
left the PE throttled at 1.2 GHz; L2 uses 4x col-tiled strips that run
concurrently on the PE array.
"""

import numpy as np
import ml_dtypes

DIM = 16
E = DIM * DIM          # 256 experts
D_IN = DIM - 1         # 15
H = 128
B = 8192
N_CORES = 8
E_CORE = E // N_CORES  # 32
CH = 1024              # batch chunk (tile free dim)
NCH = B // CH          # 8
SUBW = 512             # matmul moving width (1 fp32 PSUM bank)
Z = 4.5                # L1 preact pre-scale (clamp point of the DVE poly)
PC0, PC1, PC2 = 2.08635576, -2.07713538, 0.98841492  # tanh(Z*u/2) poly coeffs

_prog_cache = {}
L3_MODE = "accum"  # "accum" (packed 32-expert psum accumulate) | "psc" (debug: per-quad single-row)


def _register_tanh5c():
    """Register the custom DVE op (idempotent)."""
    from concourse import dve_ops
    from concourse.dve_spec import Spec, Src0, C0, C1, C2, Zero, One, maxx, minn, lower
    from concourse.dve_uop import DveOpSpec

    name = "TANH5C_ANT"
    if name in dve_ops._SUB_OPCODE_FOR_NAME:
        return next(op for op in dve_ops.OPS if op.name == name)

    xc = maxx(minn(Src0, One), Zero - One)
    s = xc * xc
    body = ((s * C2 + C1) * s + C0) * xc

    def _ref(in0, in1, s0, s1, imm2):
        u = np.clip(in0.astype(np.float32), -1.0, 1.0)
        sq = u * u
        return ((sq * imm2 + s1) * sq + s0) * u

    spec = Spec(body=body, reference=_ref)
    row = max(dve_ops._SUB_OPCODE_FOR_NAME.values()) + 1
    dve_ops._SUB_OPCODE_FOR_NAME[name] = row
    shas = {}
    for ver in ("v3", "v4"):
        uops = lower(spec, ver=ver)
        shas[ver] = DveOpSpec(name=name, opcode=row, uops=uops, rd1_en=False).sha(ver)
    op = dve_ops.DveOp(name, spec, subdim=False, uops_sha=shas)
    dve_ops.OPS.append(op)
    dve_ops.CUSTOM_DVE_SPECS[name] = spec
    return op


def _act1_on_act(e, c):
    """Which L1 tiles go to ScalarE (exact tanh) instead of the DVE poly."""
    return False  # all act1 on DVE: swapping tiles to ACT costs DVE a ~1.8us bubble


def _build_program():
    if "nc" in _prog_cache:
        return _prog_cache["nc"]

    import concourse.mybir as mybir
    import concourse.tile as tile
    from concourse import bacc

    F32 = mybir.dt.float32
    BF16 = mybir.dt.bfloat16
    TANH = mybir.ActivationFunctionType.Tanh
    dveop = _register_tanh5c()

    nc = bacc.Bacc()

    xT = nc.declare_dram_parameter("xT", [128, B], BF16, isOutput=False)
    w1p = nc.declare_dram_parameter("w1p", [128, E_CORE * H], BF16, isOutput=False)
    w2p = nc.declare_dram_parameter("w2p", [H, E_CORE * H], BF16, isOutput=False)
    w3p = nc.declare_dram_parameter("w3p", [H, E_CORE * 32], BF16, isOutput=False)
    c2p = nc.declare_dram_parameter("c2p", [H, E_CORE], F32, isOutput=False)
    out = nc.declare_dram_parameter("out", [E_CORE, B], BF16, isOutput=True)

    with tile.TileContext(nc) as tc:
        with (
            tc.tile_pool(name="const", bufs=1) as const,
            tc.tile_pool(name="t1p", bufs=5) as t1p,
            tc.tile_pool(name="t2p", bufs=8) as t2p,
            tc.tile_pool(name="ocp", bufs=3) as ocp,
            tc.tile_pool(name="pp", bufs=3, space="PSUM") as pp,
            tc.tile_pool(name="p3", bufs=1, space="PSUM") as p3,
        ):
            xT_s = const.tile([128, B], BF16, tag="xT")
            w1s = const.tile([128, E_CORE * H], BF16, tag="w1")
            w2s = const.tile([H, E_CORE * H], BF16, tag="w2")
            w3s = const.tile([H, E_CORE * 32], BF16, tag="w3")
            c2s = const.tile([H, E_CORE], F32, tag="c2")
            zrow = const.tile([1, H], BF16, tag="zrow")
            nc.vector.memset(zrow[:], 0.0)
            # priority-ordered input loads: what iteration 0 needs goes first
            nc.sync.dma_start(out=w1s[:, 0:4 * H], in_=w1p[:, 0:4 * H])
            nc.sync.dma_start(out=xT_s[:, 0:CH], in_=xT[:, 0:CH])
            nc.sync.dma_start(out=w2s[:, 0:2 * H], in_=w2p[:, 0:2 * H])
            nc.sync.dma_start(out=c2s[:], in_=c2p[:])
            nc.sync.dma_start(out=w3s[:], in_=w3p[:])
            for q in range(NCH - 1):
                sl = slice(CH + q * CH, CH + (q + 1) * CH)
                nc.sync.dma_start(out=xT_s[:, sl], in_=xT[:, sl])
            nc.sync.dma_start(out=w1s[:, 4 * H:], in_=w1p[:, 4 * H:])
            for q in range(5):
                lo = 2 * H + q * 6 * H
                hi = min(lo + 6 * H, E_CORE * H)
                nc.sync.dma_start(out=w2s[:, lo:hi], in_=w2p[:, lo:hi])

            NK = NCH * E_CORE  # 128 linear (chunk, expert) iterations
            ps1 = {}           # k -> psum tile with L1 preacts (z1/Z, bf16)
            ps2 = {}           # k -> psum tile with L2 preacts (z2', bf16)
            t1 = {}
            t2 = {}
            ps3 = None

            for k in range(NK + 2):
                c, e = k // E_CORE, k % E_CORE
                # --- L1 matmuls for iter k, first on the PE queue ---
                if k < NK:
                    p = pp.tile([128, CH], F32, tag="ps")
                    # full 128-row contraction (rows 16..127 are zero): same
                    # cost (PE time = moving cols) but counts as full PE
                    # activity for the HAM clock-gate, keeping the PE at 2.4GHz
                    for s2_ in range(2):
                        nc.tensor.matmul(
                            p[:, SUBW * s2_:SUBW * (s2_ + 1)],
                            w1s[:, H * e:H * (e + 1)],
                            xT_s[:, CH * c + SUBW * s2_:CH * c + SUBW * (s2_ + 1)],
                            start=True,
                            stop=True,
                        )
                    ps1[k] = p
                # --- act2 for iter k-2 (leads the ACT queue: it frees the
                # pool slot that iter k's L1 reuses) ---
                k2 = k - 2
                quad_ready = False
                if 0 <= k2 < NK:
                    c2, e2 = k2 // E_CORE, k2 % E_CORE
                    t = t2p.tile([128, CH], BF16, tag="t2")
                    nc.scalar.activation(
                        t[:], ps2[k2][:], TANH,
                        bias=c2s[:, e2:e2 + 1], scale=0.5,
                    )
                    t2[k2] = t
                    del ps2[k2]
                    quad_ready = (e2 % 4 == 3)
                # --- act1 for iter k-1 ---
                k1 = k - 1
                if 0 <= k1 < NK:
                    c1, e1 = k1 // E_CORE, k1 % E_CORE
                    t = t1p.tile([128, CH], BF16, tag="t1")
                    if _act1_on_act(e1, c1):
                        nc.scalar.activation(t[:], ps1[k1][:], TANH, scale=Z / 2)
                    else:
                        nc.vector._custom_dve(
                            dveop, out=t[:], in0=ps1[k1][:],
                            s0=PC0, s1=PC1, imm2=PC2,
                        )
                    t1[k1] = t
                    del ps1[k1]
                    # L2 matmuls (col-tiled 4x: strips run concurrently)
                    p = pp.tile([128, CH], F32, tag="ps")
                    for s2_ in range(2):
                        for j in range(4):
                            nc.tensor.matmul(
                                p[32 * j:32 * (j + 1),
                                  SUBW * s2_:SUBW * (s2_ + 1)],
                                w2s[:, H * e1 + 32 * j:H * e1 + 32 * (j + 1)],
                                t[:, SUBW * s2_:SUBW * (s2_ + 1)],
                                start=True,
                                stop=True,
                                tile_position=(0, 32 * j),
                            )
                    ps2[k1] = p
                # --- L3 quad (after L2 so PE stays ordered) ---
                if quad_ready:
                    if ps3 is None:
                        ps3 = p3.tile([128, CH], F32, tag="ps3")
                        for s2_ in range(2):
                            # zero-init: sets has_written for the whole
                            # region so all expert MMs accumulate
                            nc.tensor.matmul(
                                ps3[:, SUBW * s2_:SUBW * (s2_ + 1)],
                                zrow[:],
                                xT_s[0:1, SUBW * s2_:SUBW * (s2_ + 1)],
                                start=True,
                                stop=False,
                            )
                    for s2_ in range(2):
                        for eq in range(e2 - 3, e2 + 1):
                            j = eq % 4
                            nc.tensor.matmul(
                                ps3[32 * j:32 * j + 32,
                                    SUBW * s2_:SUBW * (s2_ + 1)],
                                w3s[:, 32 * eq:32 * (eq + 1)],
                                t2[k2 - 3 + (eq - (e2 - 3))][
                                    :, SUBW * s2_:SUBW * (s2_ + 1)],
                                start=False,
                                stop=(eq == E_CORE - 1),
                                tile_position=(0, 32 * j),
                            )
                    for eq in range(e2 - 3, e2 + 1):
                        del t2[k2 - 3 + (eq - (e2 - 3))]
                    if e2 == E_CORE - 1:
                        o = ocp.tile([128, CH], BF16, tag="oc")
                        nc.scalar.copy(o[:], ps3[:])
                        ps3 = None
                        for j in range(4):
                            nc.sync.dma_start(
                                out=out[8 * j:8 * j + 8,
                                        CH * c2:CH * (c2 + 1)],
                                in_=o[32 * j:32 * j + 8, :],
                            )
    nc.finalize()
    _prog_cache["nc"] = nc
    return nc


def _prep_inputs(x_batch, W1, b1, W2, b2, W3):
    """Host-side shard + layout prep. Returns per-core input maps."""
    bf = ml_dtypes.bfloat16
    # x~ = [x; 1] transposed, replicated in 4 row-bands of 32
    xT1 = np.concatenate(
        [x_batch.T, np.ones((1, B), np.float32)], axis=0
    )  # [16, B]
    xT = np.zeros((128, B), dtype=bf)
    for kb in range(4):
        xT[32 * kb:32 * kb + D_IN + 1] = xT1.astype(bf)
    in_maps = []
    for cid in range(N_CORES):
        sl = slice(cid * E_CORE, (cid + 1) * E_CORE)
        W1c, b1c = W1[sl], b1[sl]          # [32,15,128], [32,128]
        W2c, b2c = W2[sl], b2[sl]          # [32,128,128], [32,128]
        W3c = W3[sl, :, 0]                 # [32,128]
        w1pk = np.zeros((128, E_CORE * H), dtype=bf)
        for e in range(E_CORE):
            blk = (np.concatenate(
                [W1c[e], b1c[e][None, :]], axis=0
            ) / Z).astype(bf)  # [16,128]
            w1pk[0:16, H * e:H * (e + 1)] = blk
        w2pk = np.ascontiguousarray(
            (0.5 * W2c).transpose(1, 0, 2).reshape(H, E_CORE * H)
        ).astype(bf)
        c2pk = np.ascontiguousarray(
            ((b2c + 0.5 * W2c.sum(axis=1)) / 2.0).T
        ).astype(np.float32)               # [128, 32]
        w3pk = np.zeros((H, E_CORE * 32), dtype=bf)
        for e in range(E_CORE):
            w3pk[:, 32 * e + e // 4] = (0.5 * W3c[e]).astype(bf)
        in_maps.append(
            {"xT": xT, "w1p": w1pk, "w2p": w2pk, "w3p": w3pk, "c2p": c2pk}
        )
    return in_maps


def run(x_batch, W1, b1, W2, b2, W3, b3, trace=False):
    """Run on 8 NeuronCores; returns (output [B,16,16] f32, BassKernelResults)."""
    from concourse.bass_utils import run_bass_kernel_spmd

    nc = _build_program()
    x_batch = np.asarray(x_batch, dtype=np.float32)
    W1 = np.asarray(W1, dtype=np.float32)
    b1 = np.asarray(b1, dtype=np.float32)
    W2 = np.asarray(W2, dtype=np.float32)
    b2 = np.asarray(b2, dtype=np.float32)
    W3 = np.asarray(W3, dtype=np.float32)
    b3 = np.asarray(b3, dtype=np.float32)
    in_maps = _prep_inputs(x_batch, W1, b1, W2, b2, W3)
    res = run_bass_kernel_spmd(
        nc, in_maps, core_ids=list(range(N_CORES)), trace=trace
    )
    # device row 8*(e%4) + e//4  ->  expert e
    if L3_MODE == "psc":
        perm = np.arange(E_CORE)
    else:
        perm = np.array([8 * (e % 4) + e // 4 for e in range(E_CORE)])
    outs = []
    for cid in range(N_CORES):
        o = np.asarray(res.results[cid]["out"]).astype(np.float32)[perm]
        outs.append(o)
    out_full = np.concatenate(outs, axis=0)  # [E, B]
    c3 = b3[:, 0] + 0.5 * W3[:, :, 0].sum(axis=1)
    out_full = out_full + c3[:, None]
    return out_full.T.reshape(B, DIM, DIM).astype(np.float32), res


def kernel(x_batch, W1, b1, W2, b2, W3, b3):
    out, _ = run(x_batch, W1, b1, W2, b2, W3, b3, trace=False)
    return out


if __name__ == "__main__":
    rng = np.random.default_rng(0)
    ins = {
        "x_batch": rng.standard_normal((B, D_IN)).astype(np.float32),
        "W1": (rng.standard_normal((E, D_IN, H)) / np.sqrt(D_IN)).astype(np.float32),
        "b1": (rng.standard_normal((E, H)) / np.sqrt(D_IN)).astype(np.float32),
        "W2": (rng.standard_normal((E, H, H)) / np.sqrt(H)).astype(np.float32),
        "b2": (rng.standard_normal((E, H)) / np.sqrt(H)).astype(np.float32),
        "W3": (rng.standard_normal((E, H, 1)) / np.sqrt(H)).astype(np.float32),
        "b3": (rng.standard_normal((E, 1)) / np.sqrt(H)).astype(np.float32),
    }
    out = kernel(**ins)
    print("kernel ran, out shape:", out.shape, out.dtype)
